# revision 6
# baseline (speedup 1.0000x reference)
"""Adaptive embedding (4-bucket) lookup + projection on 8 TRN2 NeuronCores.

Strategy: the device program is a pure streaming GEMM; all index work is host
side (the baseline already host-gathered unique rows — this gathers the token
rows directly and uploads dense d-major matrices, removing the Q7 indirect-DMA
descriptor generation and all PE transposes from the critical path).

  Host: bucket tokens by table.  For each table upload eT = emb[rows].T
        (d on partitions) in bf16, plus the projection pre-transposed and
        pre-scaled by sqrt(D).  Work split across the 8 cores:
          t0 (d=1024): proj0 dout-halves x token-quarters (2-way model
              parallel cuts the 2MB proj0 load to 1MB/core)
          t1 (d=256):  same 2-way split
          t2 (d=64):   token-parallel; two token halves on SBUF partitions
              0-63 / 64-127 as PE row-tiles
          t3 (d=16):   token-parallel, row-tiles at partitions 0-15 / 32-47
  Core: out[dout_block, tok] = projT_block.T @ eT accumulated over k-tiles
        in PSUM; DVE/ACT alternate evacuating PSUM (one strided copy per
        2-bank group).  t2/t3 outputs are cast to fp8e4 during the evac
        (their share of the output norm is ~31%, so the ~2.9% fp8
        quantization RMS costs ~1.5% global rel err vs the 2e-2 budget)
        which halves the dominant store traffic.  t0 accumulation (k-tiles
        0-3) is interleaved into the t2 stream so PSUM production never
        outruns the DVE/ACT drain rate and the PE stays HAM-warm.
  Host: transpose dout-major results back to token order, upcast to f32.

DMA per core: ~2.6MB loads + ~2.4MB stores.
"""

import os
import sys

import numpy as np

for _p in ("/opt/trn_rl_repo",):
    if _p not in sys.path:
        sys.path.insert(0, _p)

import ml_dtypes

BF16 = ml_dtypes.bfloat16
FP8 = ml_dtypes.float8_e4m3

N_TOKEN = 267735
CUTS = (0, 20000, 40000, 200000, N_TOKEN)
D_OUT = 1024
EMB_SCALE = float(D_OUT) ** 0.5
N_CORES = 8
P = 128

_PROGRAM_CACHE = {}
LAST_RESULTS = None  # BassKernelResults of the most recent run (for profiling)


def _chunks(n, m=512):
    out = []
    c = 0
    while c < n:
        out.append((c, min(c + m, n)))
        c += m
    return out


def _build_program(n0q, n1q, n2h, n3h):
    import concourse.bacc as bacc
    import concourse.mybir as mybir
    import concourse.tile as tile

    dt = mybir.dt
    nc = bacc.Bacc("TRN2", target_bir_lowering=False, debug=False)

    n2c, n3c = 2 * n2h, 2 * n3h
    assert n2h <= 1024 and n3h <= 512 and n0q <= 512 and n1q <= 512

    warm = nc.dram_tensor("warm", [P, P], dt.bfloat16, kind="ExternalInput")
    in3 = nc.dram_tensor("in3", [48, 1024 + n3h], dt.bfloat16,
                         kind="ExternalInput")
    in2 = nc.dram_tensor("in2", [P, 1024 + n2h], dt.bfloat16,
                         kind="ExternalInput")
    # in0 halves: [p0 k0-3 | e0 k0-3] then [p0 k4-7 | e0 k4-7]
    x0h = 2048 + 4 * n0q
    in0 = nc.dram_tensor("in0", [P, 2 * x0h], dt.bfloat16,
                         kind="ExternalInput")
    in1 = nc.dram_tensor("in1", [P, 1024 + 2 * n1q], dt.bfloat16,
                         kind="ExternalInput")

    o0 = nc.dram_tensor("o0", [P, 4, n0q], dt.bfloat16, kind="ExternalOutput")
    o1 = nc.dram_tensor("o1", [P, 4, n1q], dt.bfloat16, kind="ExternalOutput")
    o2 = nc.dram_tensor("o2", [P, 8, n2c], dt.float8e4, kind="ExternalOutput")
    o3 = nc.dram_tensor("o3", [P, 8, 2, n3h], dt.float8e4,
                        kind="ExternalOutput")

    with tile.TileContext(nc) as tc:
        with (
            tc.tile_pool(name="io", bufs=1) as io,
            tc.tile_pool(name="psb", bufs=2, space="PSUM") as ppb,
            tc.tile_pool(name="pss", bufs=4, space="PSUM") as pps,
        ):
            # --- loads; SP and ACT HWDGE rings dispatch in parallel, so
            # the urgent in0 half A streams concurrently with in3/in2
            warm_sb = io.tile([P, P], dt.bfloat16, tag="warm")
            nc.sync.dma_start(warm_sb[:], warm[:])
            in3_sb = io.tile([48, 1024 + n3h], dt.bfloat16, tag="in3")
            nc.sync.dma_start(in3_sb[:], in3[:])
            in0_sb = io.tile([P, 2 * x0h], dt.bfloat16, tag="in0")
            nc.scalar.dma_start(in0_sb[:, 0:x0h], in0[:, 0:x0h])
            in2_sb = io.tile([P, 1024 + n2h], dt.bfloat16, tag="in2")
            nc.sync.dma_start(in2_sb[:], in2[:])
            nc.scalar.dma_start(in0_sb[:, x0h:], in0[:, x0h:])
            in1_sb = io.tile([P, 1024 + 2 * n1q], dt.bfloat16, tag="in1")
            nc.scalar.dma_start(in1_sb[:], in1[:])

            st0 = io.tile([P, 4, n0q], dt.bfloat16, tag="st0")
            st1 = io.tile([P, 4, n1q], dt.bfloat16, tag="st1")
            st2 = io.tile([P, 8, n2c], dt.float8e4, tag="st2")
            st3 = io.tile([P, 8, 2, n3h], dt.float8e4, tag="st3")

            flip = [0]

            def evac(dst, ps):
                # DVE is ~1.4x ACT on copies; give it 4 of every 7
                if flip[0] % 7 < 4:
                    nc.vector.tensor_copy(dst, ps)
                else:
                    nc.scalar.copy(dst, ps)
                flip[0] += 1

            # --- PE warm-up fillers (HAM un-throttles after ~3.4us busy)
            for i in range(14):
                psw = pps.tile([P, 512], dt.float32, tag="ps", name=f"w{i}")
                nc.tensor.matmul(psw[:, 0:P], warm_sb[:], warm_sb[:],
                                 start=True, stop=True)

            # --- t3: d=16, row-tiles at partitions 0-15 / 32-47; both
            # tiles of a block share one 2-bank PSUM tile -> single evac
            for s in range(8):
                ps = ppb.tile([P, 2, 512], dt.float32, tag="pb",
                              name=f"ps3_{s}")
                for i, base in enumerate((0, 32)):
                    nc.tensor.matmul(
                        ps[:, i, 0:n3h],
                        in3_sb[base:base + 16, s * P:(s + 1) * P],
                        in3_sb[base:base + 16, 1024:1024 + n3h],
                        start=True, stop=True)
                evac(st3[:, s, :, :], ps[:, :, 0:n3h])
            nc.sync.dma_start(o3[:], st3[:])

            # --- t0 helpers: phase A (k 0-3) accumulates into a held PSUM
            # bank, phase B (k 4-7) finishes + evacs
            def p0_ap(k, s):
                base = (k // 4) * x0h
                return in0_sb[:, base + ((k % 4) * 4 + s) * P:
                              base + ((k % 4) * 4 + s + 1) * P]

            def e0_ap(k):
                base = (k // 4) * x0h + 2048
                return in0_sb[:, base + (k % 4) * n0q:
                              base + (k % 4 + 1) * n0q]

            ps0 = {}

            def t0_group(s, ks):
                ps = ps0.get(s)
                if ps is None:
                    ps = pps.tile([P, 512], dt.float32, tag="ps",
                                  name=f"ps0_{s}")
                    ps0[s] = ps
                for k in ks:
                    nc.tensor.matmul(ps[:, 0:n0q], p0_ap(k, s), e0_ap(k),
                                     start=(k == 0), stop=(k == 7))
                if ks[-1] == 7:
                    evac(st0[:, s, :], ps[:, 0:n0q])

            # in0 half A arrives right after in3: run all t0 phase A now
            # (no evacs -> lets DVE/ACT drain the t3 backlog meanwhile)
            for s in range(4):
                t0_group(s, [0, 1, 2, 3])

            # --- t2: d=64, row-tiles at partitions 0-63 / 64-127, one
            # 2-bank PSUM tile per (block, tile) -> single n2h-col evac.
            # t0 phase-B groups interleave once in0 half B has landed.
            for s in range(8):
                for base, off in ((0, 0), (64, n2h)):
                    ps = ppb.tile([P, 1024], dt.float32, tag="pb",
                                  name=f"ps2_{s}_{base}")
                    for c0, c1 in _chunks(n2h):
                        nc.tensor.matmul(
                            ps[:, c0:c1],
                            in2_sb[base:base + 64, s * P:(s + 1) * P],
                            in2_sb[base:base + 64, 1024 + c0:1024 + c1],
                            start=True, stop=True)
                    evac(st2[:, s, off:off + n2h], ps[:, 0:n2h])
                if s == 3:
                    nc.sync.dma_start(o2[:, 0:4, :], st2[:, 0:4, :])
                if s == 4:
                    t0_group(0, [4, 5, 6, 7])
                    t0_group(1, [4, 5, 6, 7])
                if s == 6:
                    t0_group(2, [4, 5, 6, 7])
                    t0_group(3, [4, 5, 6, 7])
            nc.sync.dma_start(o2[:, 4:8, :], st2[:, 4:8, :])
            nc.sync.dma_start(o0[:], st0[:])

            # --- t1: d=256, 2 k-tiles, dout-half shard
            for s in range(4):
                ps = ppb.tile([P, 1024], dt.float32, tag="pb",
                              name=f"ps1_{s}")
                for k in range(2):
                    nc.tensor.matmul(
                        ps[:, 0:n1q],
                        in1_sb[:, (k * 4 + s) * P:(k * 4 + s + 1) * P],
                        in1_sb[:, 1024 + k * n1q:1024 + (k + 1) * n1q],
                        start=(k == 0), stop=(k == 1))
                evac(st1[:, s, :], ps[:, 0:n1q])
            nc.sync.dma_start(o1[:], st1[:])

    nc.finalize()
    return nc


def _pad_cols(a, n):
    if a.shape[1] == n:
        return a
    out = np.zeros((a.shape[0], n), a.dtype)
    out[:, :a.shape[1]] = a
    return out


def kernel(inp, emb0, emb1, emb2, emb3, proj0, proj1, proj2, proj3):
    global LAST_RESULTS
    from concourse.bass_utils import run_bass_kernel_spmd

    flat = np.asarray(inp).reshape(-1).astype(np.int64)
    T = flat.shape[0]
    cuts = np.asarray(CUTS)
    tblid = np.searchsorted(cuts[1:], flat, side="right")
    embs = [np.asarray(e, np.float32) for e in (emb0, emb1, emb2, emb3)]
    projTs = [
        np.ascontiguousarray((np.asarray(p, np.float32) * EMB_SCALE).T)
        for p in (proj0, proj1, proj2, proj3)
    ]

    pos = {}
    loc = {}
    for t in range(4):
        pos[t] = np.nonzero(tblid == t)[0]
        loc[t] = flat[pos[t]] - cuts[t]

    n0q = max(1, -(-len(pos[0]) // 4))
    n1q = max(1, -(-len(pos[1]) // 4))
    n2h = max(1, -(-len(pos[2]) // 16))
    n3h = max(1, -(-len(pos[3]) // 16))
    n2c, n3c = 2 * n2h, 2 * n3h

    key = (n0q, n1q, n2h, n3h)
    nc = _PROGRAM_CACHE.get(key)
    if nc is None:
        nc = _build_program(*key)
        _PROGRAM_CACHE[key] = nc

    warm_np = np.zeros((P, P), BF16)

    e0_q = []
    for q in range(4):
        et = embs[0][loc[0][q::4]].T  # [1024, n]
        et = _pad_cols(et, n0q).reshape(8, P, n0q)
        e0_q.append(np.ascontiguousarray(et.transpose(1, 0, 2)).astype(BF16))
    pk0 = projTs[0].reshape(8, P, 8, P)  # [k, d_part, s_glob, c]
    p0_h = [
        np.ascontiguousarray(
            pk0[:, :, h * 4:(h + 1) * 4, :].transpose(1, 0, 2, 3)
        ).astype(BF16)
        for h in range(2)
    ]

    e1_q = []
    for q in range(4):
        et = embs[1][loc[1][q::4]].T  # [256, n]
        et = _pad_cols(et, n1q).reshape(2, P, n1q)
        e1_q.append(np.ascontiguousarray(et.transpose(1, 0, 2)).astype(BF16))
    pk1 = projTs[1].reshape(2, P, 8, P)
    p1_h = [
        np.ascontiguousarray(
            pk1[:, :, h * 4:(h + 1) * 4, :].transpose(1, 0, 2, 3)
        ).astype(BF16)
        for h in range(2)
    ]

    pk2 = projTs[2].reshape(64, 8 * P)
    p2 = np.concatenate([pk2, pk2], axis=0).astype(BF16)  # [128, 1024]
    pk3 = projTs[3].reshape(16, 8 * P)
    p3 = np.zeros((48, 8 * P), np.float32)
    p3[0:16] = pk3
    p3[32:48] = pk3
    p3 = p3.astype(BF16)

    in_maps = []
    core_meta = []
    for k in range(N_CORES):
        q, h = k // 2, k % 2

        p0 = p0_h[h]
        e0 = e0_q[q]
        in0 = np.concatenate([
            p0[:, 0:4].reshape(P, -1), e0[:, 0:4].reshape(P, -1),
            p0[:, 4:8].reshape(P, -1), e0[:, 4:8].reshape(P, -1),
        ], axis=1)

        in1 = np.concatenate([
            p1_h[h].reshape(P, -1), e1_q[q].reshape(P, -1)
        ], axis=1)

        rows2 = loc[2][k::8]
        nA2 = min(len(rows2), n2h)
        eA = _pad_cols(embs[2][rows2[:nA2]].T, n2h)
        eB = _pad_cols(embs[2][rows2[nA2:]].T, n2h)
        in2 = np.concatenate(
            [p2, np.concatenate([eA, eB], axis=0).astype(BF16)], axis=1)

        rows3 = loc[3][k::8]
        nA3 = min(len(rows3), n3h)
        e3 = np.zeros((48, n3h), np.float32)
        e3[0:16, :nA3] = embs[3][rows3[:nA3]].T
        e3[32:48, :len(rows3) - nA3] = embs[3][rows3[nA3:]].T
        in3 = np.concatenate([p3, e3.astype(BF16)], axis=1)

        in_maps.append({
            "warm": warm_np, "in0": np.ascontiguousarray(in0),
            "in1": np.ascontiguousarray(in1),
            "in2": np.ascontiguousarray(in2),
            "in3": np.ascontiguousarray(in3),
        })
        core_meta.append((nA2, nA3))

    trace = bool(os.environ.get("KERNEL_TRACE"))
    res = run_bass_kernel_spmd(nc, in_maps, core_ids=list(range(N_CORES)),
                               trace=trace)
    LAST_RESULTS = res

    out = np.empty((T, D_OUT), np.float32)

    for k in range(N_CORES):
        q, h = k // 2, k % 2
        r = res.results[k]
        for t, name in ((0, "o0"), (1, "o1")):
            positions = pos[t][q::4]
            n = len(positions)
            if n:
                dat = np.asarray(r[name])[:, :, :n].astype(np.float32)
                out[np.ix_(positions,
                           np.arange(h * 512, h * 512 + 512))] = (
                    dat.transpose(2, 1, 0).reshape(n, 512))
        nA2, nA3 = core_meta[k]
        p2k = pos[2][k::8]
        ob = np.asarray(r["o2"]).astype(np.float32)  # [128, 8, n2c]
        if nA2:
            out[p2k[:nA2]] = ob[:, :, :nA2].transpose(2, 1, 0).reshape(
                nA2, D_OUT)
        nB = len(p2k) - nA2
        if nB > 0:
            out[p2k[nA2:]] = ob[:, :, n2h:n2h + nB].transpose(2, 1, 0).reshape(
                nB, D_OUT)
        p3k = pos[3][k::8]
        ob = np.asarray(r["o3"]).astype(np.float32)  # [128, 8, 2, n3h]
        if nA3:
            out[p3k[:nA3]] = ob[:, :, 0, :nA3].transpose(2, 1, 0).reshape(
                nA3, D_OUT)
        nB = len(p3k) - nA3
        if nB > 0:
            out[p3k[nA3:]] = ob[:, :, 1, :nB].transpose(2, 1, 0).reshape(
                nB, D_OUT)

    return out.reshape(*np.asarray(inp).shape, D_OUT)


# revision 7
# speedup vs baseline: 1.2620x; 1.2620x over previous
"""Adaptive embedding (4-bucket) lookup + projection on 8 TRN2 NeuronCores.

Strategy: the device program is a pure streaming GEMM; all index work is host
side (the baseline already host-gathered unique rows — this gathers the token
rows directly and uploads dense d-major matrices, removing the Q7 indirect-DMA
descriptor generation and all PE transposes from the critical path).

  Host: bucket tokens by table.  For each table upload eT = emb[rows].T
        (d on partitions) in bf16, plus the projection pre-transposed and
        pre-scaled by sqrt(D).  Work split across the 8 cores:
          t0 (d=1024): proj0 dout-halves x token-quarters (2-way model
              parallel cuts the 2MB proj0 load to 1MB/core)
          t1 (d=256):  same 2-way split
          t2 (d=64):   token-parallel; two token halves on SBUF partitions
              0-63 / 64-127 as PE row-tiles
          t3 (d=16):   token-parallel, row-tiles at partitions 0-15 / 32-47
  Core: out[dout_block, tok] = projT_block.T @ eT accumulated over k-tiles
        in PSUM; DVE/ACT alternate evacuating PSUM (one strided copy per
        2-bank group).  t2/t3 outputs are cast to fp8e4 during the evac
        (their share of the output norm is ~31%, so the ~2.9% fp8
        quantization RMS costs ~1.5% global rel err vs the 2e-2 budget)
        which halves the dominant store traffic.  t0 accumulation (k-tiles
        0-3) is interleaved into the t2 stream so PSUM production never
        outruns the DVE/ACT drain rate and the PE stays HAM-warm.
  Host: transpose dout-major results back to token order, upcast to f32.

DMA per core: ~2.6MB loads + ~2.4MB stores.
"""

import os
import sys

import numpy as np

for _p in ("/opt/trn_rl_repo",):
    if _p not in sys.path:
        sys.path.insert(0, _p)

import ml_dtypes

BF16 = ml_dtypes.bfloat16
FP8 = ml_dtypes.float8_e4m3

N_TOKEN = 267735
CUTS = (0, 20000, 40000, 200000, N_TOKEN)
D_OUT = 1024
EMB_SCALE = float(D_OUT) ** 0.5
N_CORES = 8
P = 128

_PROGRAM_CACHE = {}
LAST_RESULTS = None  # BassKernelResults of the most recent run (for profiling)


def _chunks(n, m=512):
    out = []
    c = 0
    while c < n:
        out.append((c, min(c + m, n)))
        c += m
    return out


def _build_program(n0q, n1q, n2h, n3h):
    import concourse.bacc as bacc
    import concourse.mybir as mybir
    import concourse.tile as tile

    dt = mybir.dt
    nc = bacc.Bacc("TRN2", target_bir_lowering=False, debug=False)

    n2c, n3c = 2 * n2h, 2 * n3h
    assert n2h <= 1024 and n3h <= 512 and n0q <= 512 and n1q <= 512

    warm = nc.dram_tensor("warm", [P, P], dt.bfloat16, kind="ExternalInput")
    in3 = nc.dram_tensor("in3", [48, 1024 + n3h], dt.bfloat16,
                         kind="ExternalInput")
    in2 = nc.dram_tensor("in2", [P, 1024 + n2h], dt.bfloat16,
                         kind="ExternalInput")
    # in0 halves: [p0 k0-3 | e0 k0-3] then [p0 k4-7 | e0 k4-7]
    x0h = 2048 + 4 * n0q
    in0 = nc.dram_tensor("in0", [P, 2 * x0h], dt.bfloat16,
                         kind="ExternalInput")
    in1 = nc.dram_tensor("in1", [P, 1024 + 2 * n1q], dt.bfloat16,
                         kind="ExternalInput")

    o0 = nc.dram_tensor("o0", [P, 4, n0q], dt.bfloat16, kind="ExternalOutput")
    o1 = nc.dram_tensor("o1", [P, 4, n1q], dt.bfloat16, kind="ExternalOutput")
    o2 = nc.dram_tensor("o2", [P, 8, n2c], dt.float8e4, kind="ExternalOutput")
    o3 = nc.dram_tensor("o3", [P, 8, 2, n3h], dt.float8e4,
                        kind="ExternalOutput")

    with tile.TileContext(nc) as tc:
        with (
            tc.tile_pool(name="io", bufs=1) as io,
            tc.tile_pool(name="psum", bufs=8, space="PSUM") as pp,
        ):
            # --- loads, all on the SP HWDGE ring (ACT stays evac-only)
            warm_sb = io.tile([P, P], dt.bfloat16, tag="warm")
            nc.sync.dma_start(warm_sb[:], warm[:])
            in3_sb = io.tile([48, 1024 + n3h], dt.bfloat16, tag="in3")
            nc.sync.dma_start(in3_sb[:], in3[:])
            in0_sb = io.tile([P, 2 * x0h], dt.bfloat16, tag="in0")
            nc.sync.dma_start(in0_sb[:, 0:x0h], in0[:, 0:x0h])
            nc.sync.dma_start(in0_sb[:, x0h:], in0[:, x0h:])
            in2_sb = io.tile([P, 1024 + n2h], dt.bfloat16, tag="in2")
            nc.sync.dma_start(in2_sb[:], in2[:])
            in1_sb = io.tile([P, 1024 + 2 * n1q], dt.bfloat16, tag="in1")
            nc.sync.dma_start(in1_sb[:], in1[:])

            st0 = io.tile([P, 4, n0q], dt.bfloat16, tag="st0")
            st1 = io.tile([P, 4, n1q], dt.bfloat16, tag="st1")
            st2 = io.tile([P, 8, n2c], dt.float8e4, tag="st2")
            st3 = io.tile([P, 8, 2, n3h], dt.float8e4, tag="st3")

            flip = [0]

            def evac(dst, ps):
                if flip[0] % 2 == 0:
                    nc.vector.tensor_copy(dst, ps)
                else:
                    nc.scalar.copy(dst, ps)
                flip[0] += 1

            def psum(name):
                return pp.tile([P, 512], dt.float32, tag="ps", name=name)

            # --- PE warm-up fillers (HAM un-throttles after ~3.4us busy)
            for i in range(14):
                psw = psum(f"w{i}")
                nc.tensor.matmul(psw[:, 0:P], warm_sb[:], warm_sb[:],
                                 start=True, stop=True)

            # --- t3: d=16, row-tiles at partitions 0-15 / 32-47
            def t3_block(s):
                for i, base in enumerate((0, 32)):
                    ps = psum(f"ps3_{s}_{i}")
                    nc.tensor.matmul(
                        ps[:, 0:n3h],
                        in3_sb[base:base + 16, s * P:(s + 1) * P],
                        in3_sb[base:base + 16, 1024:1024 + n3h],
                        start=True, stop=True)
                    evac(st3[:, s, i, :], ps[:, 0:n3h])

            # --- t0: d=1024, 8 accumulating k-tiles per dout block
            def p0_ap(k, s):
                base = (k // 4) * x0h
                return in0_sb[:, base + ((k % 4) * 4 + s) * P:
                              base + ((k % 4) * 4 + s + 1) * P]

            def e0_ap(k):
                base = (k // 4) * x0h + 2048
                return in0_sb[:, base + (k % 4) * n0q:
                              base + (k % 4 + 1) * n0q]

            def t0_block(s):
                ps = psum(f"ps0_{s}")
                for k in range(8):
                    nc.tensor.matmul(ps[:, 0:n0q], p0_ap(k, s), e0_ap(k),
                                     start=(k == 0), stop=(k == 7))
                evac(st0[:, s, :], ps[:, 0:n0q])

            # --- t2: d=64, row-tiles at partitions 0-63 / 64-127
            def t2_block(s):
                for base, off in ((0, 0), (64, n2h)):
                    for c0, c1 in _chunks(n2h):
                        ps = psum(f"ps2_{s}_{base}_{c0}")
                        nc.tensor.matmul(
                            ps[:, 0:c1 - c0],
                            in2_sb[base:base + 64, s * P:(s + 1) * P],
                            in2_sb[base:base + 64, 1024 + c0:1024 + c1],
                            start=True, stop=True)
                        evac(st2[:, s, off + c0:off + c1], ps[:, 0:c1 - c0])

            # Interleave heavy-MM/light-evac t0 blocks with light-MM/
            # heavy-evac t3/t2 so PSUM production never outruns the
            # DVE/ACT drain rate and the PE never stalls on banks.
            for s in range(8):
                t3_block(s)
            nc.sync.dma_start(o3[:], st3[:])
            t0_block(0)
            t0_block(1)
            for s in range(4):
                t2_block(s)
            nc.sync.dma_start(o2[:, 0:4, :], st2[:, 0:4, :])
            t0_block(2)
            t0_block(3)
            nc.sync.dma_start(o0[:], st0[:])
            for s in range(4, 8):
                t2_block(s)
            nc.sync.dma_start(o2[:, 4:8, :], st2[:, 4:8, :])

            # --- t1: d=256, 2 k-tiles, dout-half shard
            for s in range(4):
                ps = psum(f"ps1_{s}")
                for k in range(2):
                    nc.tensor.matmul(
                        ps[:, 0:n1q],
                        in1_sb[:, (k * 4 + s) * P:(k * 4 + s + 1) * P],
                        in1_sb[:, 1024 + k * n1q:1024 + (k + 1) * n1q],
                        start=(k == 0), stop=(k == 1))
                evac(st1[:, s, :], ps[:, 0:n1q])
            nc.sync.dma_start(o1[:], st1[:])

    nc.finalize()
    return nc


def _pad_cols(a, n):
    if a.shape[1] == n:
        return a
    out = np.zeros((a.shape[0], n), a.dtype)
    out[:, :a.shape[1]] = a
    return out


def kernel(inp, emb0, emb1, emb2, emb3, proj0, proj1, proj2, proj3):
    global LAST_RESULTS
    from concourse.bass_utils import run_bass_kernel_spmd

    flat = np.asarray(inp).reshape(-1).astype(np.int64)
    T = flat.shape[0]
    cuts = np.asarray(CUTS)
    tblid = np.searchsorted(cuts[1:], flat, side="right")
    embs = [np.asarray(e, np.float32) for e in (emb0, emb1, emb2, emb3)]
    projTs = [
        np.ascontiguousarray((np.asarray(p, np.float32) * EMB_SCALE).T)
        for p in (proj0, proj1, proj2, proj3)
    ]

    pos = {}
    loc = {}
    for t in range(4):
        pos[t] = np.nonzero(tblid == t)[0]
        loc[t] = flat[pos[t]] - cuts[t]

    n0q = max(1, -(-len(pos[0]) // 4))
    n1q = max(1, -(-len(pos[1]) // 4))
    n2h = max(1, -(-len(pos[2]) // 16))
    n3h = max(1, -(-len(pos[3]) // 16))
    n2c, n3c = 2 * n2h, 2 * n3h

    key = (n0q, n1q, n2h, n3h)
    nc = _PROGRAM_CACHE.get(key)
    if nc is None:
        nc = _build_program(*key)
        _PROGRAM_CACHE[key] = nc

    warm_np = np.zeros((P, P), BF16)

    e0_q = []
    for q in range(4):
        et = embs[0][loc[0][q::4]].T  # [1024, n]
        et = _pad_cols(et, n0q).reshape(8, P, n0q)
        e0_q.append(np.ascontiguousarray(et.transpose(1, 0, 2)).astype(BF16))
    pk0 = projTs[0].reshape(8, P, 8, P)  # [k, d_part, s_glob, c]
    p0_h = [
        np.ascontiguousarray(
            pk0[:, :, h * 4:(h + 1) * 4, :].transpose(1, 0, 2, 3)
        ).astype(BF16)
        for h in range(2)
    ]

    e1_q = []
    for q in range(4):
        et = embs[1][loc[1][q::4]].T  # [256, n]
        et = _pad_cols(et, n1q).reshape(2, P, n1q)
        e1_q.append(np.ascontiguousarray(et.transpose(1, 0, 2)).astype(BF16))
    pk1 = projTs[1].reshape(2, P, 8, P)
    p1_h = [
        np.ascontiguousarray(
            pk1[:, :, h * 4:(h + 1) * 4, :].transpose(1, 0, 2, 3)
        ).astype(BF16)
        for h in range(2)
    ]

    pk2 = projTs[2].reshape(64, 8 * P)
    p2 = np.concatenate([pk2, pk2], axis=0).astype(BF16)  # [128, 1024]
    pk3 = projTs[3].reshape(16, 8 * P)
    p3 = np.zeros((48, 8 * P), np.float32)
    p3[0:16] = pk3
    p3[32:48] = pk3
    p3 = p3.astype(BF16)

    in_maps = []
    core_meta = []
    for k in range(N_CORES):
        q, h = k // 2, k % 2

        p0 = p0_h[h]
        e0 = e0_q[q]
        in0 = np.concatenate([
            p0[:, 0:4].reshape(P, -1), e0[:, 0:4].reshape(P, -1),
            p0[:, 4:8].reshape(P, -1), e0[:, 4:8].reshape(P, -1),
        ], axis=1)

        in1 = np.concatenate([
            p1_h[h].reshape(P, -1), e1_q[q].reshape(P, -1)
        ], axis=1)

        rows2 = loc[2][k::8]
        nA2 = min(len(rows2), n2h)
        eA = _pad_cols(embs[2][rows2[:nA2]].T, n2h)
        eB = _pad_cols(embs[2][rows2[nA2:]].T, n2h)
        in2 = np.concatenate(
            [p2, np.concatenate([eA, eB], axis=0).astype(BF16)], axis=1)

        rows3 = loc[3][k::8]
        nA3 = min(len(rows3), n3h)
        e3 = np.zeros((48, n3h), np.float32)
        e3[0:16, :nA3] = embs[3][rows3[:nA3]].T
        e3[32:48, :len(rows3) - nA3] = embs[3][rows3[nA3:]].T
        in3 = np.concatenate([p3, e3.astype(BF16)], axis=1)

        in_maps.append({
            "warm": warm_np, "in0": np.ascontiguousarray(in0),
            "in1": np.ascontiguousarray(in1),
            "in2": np.ascontiguousarray(in2),
            "in3": np.ascontiguousarray(in3),
        })
        core_meta.append((nA2, nA3))

    trace = bool(os.environ.get("KERNEL_TRACE"))
    res = run_bass_kernel_spmd(nc, in_maps, core_ids=list(range(N_CORES)),
                               trace=trace)
    LAST_RESULTS = res

    out = np.empty((T, D_OUT), np.float32)

    for k in range(N_CORES):
        q, h = k // 2, k % 2
        r = res.results[k]
        for t, name in ((0, "o0"), (1, "o1")):
            positions = pos[t][q::4]
            n = len(positions)
            if n:
                dat = np.asarray(r[name])[:, :, :n].astype(np.float32)
                out[np.ix_(positions,
                           np.arange(h * 512, h * 512 + 512))] = (
                    dat.transpose(2, 1, 0).reshape(n, 512))
        nA2, nA3 = core_meta[k]
        p2k = pos[2][k::8]
        ob = np.asarray(r["o2"]).astype(np.float32)  # [128, 8, n2c]
        if nA2:
            out[p2k[:nA2]] = ob[:, :, :nA2].transpose(2, 1, 0).reshape(
                nA2, D_OUT)
        nB = len(p2k) - nA2
        if nB > 0:
            out[p2k[nA2:]] = ob[:, :, n2h:n2h + nB].transpose(2, 1, 0).reshape(
                nB, D_OUT)
        p3k = pos[3][k::8]
        ob = np.asarray(r["o3"]).astype(np.float32)  # [128, 8, 2, n3h]
        if nA3:
            out[p3k[:nA3]] = ob[:, :, 0, :nA3].transpose(2, 1, 0).reshape(
                nA3, D_OUT)
        nB = len(p3k) - nA3
        if nB > 0:
            out[p3k[nA3:]] = ob[:, :, 1, :nB].transpose(2, 1, 0).reshape(
                nB, D_OUT)

    return out.reshape(*np.asarray(inp).shape, D_OUT)


# revision 9
# speedup vs baseline: 1.2644x; 1.0019x over previous
"""Adaptive embedding (4-bucket) lookup + projection on 8 TRN2 NeuronCores.

Strategy: the device program is a pure streaming GEMM; all index work is host
side (the baseline already host-gathered unique rows — this gathers the token
rows directly and uploads dense d-major matrices, removing the Q7 indirect-DMA
descriptor generation and all PE transposes from the critical path).

  Host: bucket tokens by table.  For each table upload eT = emb[rows].T
        (d on partitions) in bf16, plus the projection pre-transposed and
        pre-scaled by sqrt(D).  Work split across the 8 cores:
          t0 (d=1024): proj0 dout-halves x token-quarters (2-way model
              parallel cuts the 2MB proj0 load to 1MB/core)
          t1 (d=256):  same 2-way split
          t2 (d=64):   token-parallel; two token halves on SBUF partitions
              0-63 / 64-127 as PE row-tiles
          t3 (d=16):   token-parallel, row-tiles at partitions 0-15 / 32-47
  Core: out[dout_block, tok] = projT_block.T @ eT accumulated over k-tiles
        in PSUM; DVE/ACT alternate evacuating PSUM (one strided copy per
        2-bank group).  t2/t3 outputs are cast to fp8e4 during the evac
        (their share of the output norm is ~31%, so the ~2.9% fp8
        quantization RMS costs ~1.5% global rel err vs the 2e-2 budget)
        which halves the dominant store traffic.  t0 accumulation (k-tiles
        0-3) is interleaved into the t2 stream so PSUM production never
        outruns the DVE/ACT drain rate and the PE stays HAM-warm.
  Host: transpose dout-major results back to token order, upcast to f32.

DMA per core: ~2.6MB loads + ~2.4MB stores.
"""

import os
import sys

import numpy as np

for _p in ("/opt/trn_rl_repo",):
    if _p not in sys.path:
        sys.path.insert(0, _p)

import ml_dtypes

BF16 = ml_dtypes.bfloat16
FP8 = ml_dtypes.float8_e4m3

N_TOKEN = 267735
CUTS = (0, 20000, 40000, 200000, N_TOKEN)
D_OUT = 1024
EMB_SCALE = float(D_OUT) ** 0.5
N_CORES = 8
P = 128

_PROGRAM_CACHE = {}
LAST_RESULTS = None  # BassKernelResults of the most recent run (for profiling)


def _chunks(n, m=512):
    out = []
    c = 0
    while c < n:
        out.append((c, min(c + m, n)))
        c += m
    return out


def _build_program(n0q, n1q, n2h, n3h):
    import concourse.bacc as bacc
    import concourse.mybir as mybir
    import concourse.tile as tile

    dt = mybir.dt
    nc = bacc.Bacc("TRN2", target_bir_lowering=False, debug=False)

    n2c, n3c = 2 * n2h, 2 * n3h
    assert n2h <= 1024 and n3h <= 512 and n0q <= 512 and n1q <= 512

    warm = nc.dram_tensor("warm", [P, P], dt.bfloat16, kind="ExternalInput")
    in3 = nc.dram_tensor("in3", [48, 1024 + n3h], dt.bfloat16,
                         kind="ExternalInput")
    in2 = nc.dram_tensor("in2", [P, 1024 + n2h], dt.bfloat16,
                         kind="ExternalInput")
    # in0 halves: [p0 k0-3 | e0 k0-3] then [p0 k4-7 | e0 k4-7]
    x0h = 2048 + 4 * n0q
    in0 = nc.dram_tensor("in0", [P, 2 * x0h], dt.bfloat16,
                         kind="ExternalInput")
    in1 = nc.dram_tensor("in1", [P, 1024 + 2 * n1q], dt.bfloat16,
                         kind="ExternalInput")

    o0 = nc.dram_tensor("o0", [P, 4, n0q], dt.bfloat16, kind="ExternalOutput")
    o1 = nc.dram_tensor("o1", [P, 4, n1q], dt.bfloat16, kind="ExternalOutput")
    o2 = nc.dram_tensor("o2", [P, 8, n2c], dt.float8e4, kind="ExternalOutput")
    o3 = nc.dram_tensor("o3", [P, 8, 2, n3h], dt.float8e4,
                        kind="ExternalOutput")

    with tile.TileContext(nc) as tc:
        with (
            tc.tile_pool(name="io", bufs=1) as io,
            tc.tile_pool(name="psb", bufs=3, space="PSUM") as ppb,
            tc.tile_pool(name="pss", bufs=2, space="PSUM") as pps,
        ):
            # --- loads, all on the SP HWDGE ring (ACT stays evac-only),
            # ordered by when the PE stream first needs each tensor
            warm_sb = io.tile([P, P], dt.bfloat16, tag="warm")
            nc.sync.dma_start(warm_sb[:], warm[:])
            in3_sb = io.tile([48, 1024 + n3h], dt.bfloat16, tag="in3")
            nc.sync.dma_start(in3_sb[:], in3[:])
            in0_sb = io.tile([P, 2 * x0h], dt.bfloat16, tag="in0")
            nc.sync.dma_start(in0_sb[:, 0:x0h], in0[:, 0:x0h])
            in2_sb = io.tile([P, 1024 + n2h], dt.bfloat16, tag="in2")
            nc.sync.dma_start(in2_sb[:], in2[:])
            nc.sync.dma_start(in0_sb[:, x0h:], in0[:, x0h:])
            in1_sb = io.tile([P, 1024 + 2 * n1q], dt.bfloat16, tag="in1")
            nc.sync.dma_start(in1_sb[:], in1[:])

            st0 = io.tile([P, 4, n0q], dt.bfloat16, tag="st0")
            st1 = io.tile([P, 4, n1q], dt.bfloat16, tag="st1")
            st2 = io.tile([P, 8, n2c], dt.float8e4, tag="st2")
            st3 = io.tile([P, 8, 2, n3h], dt.float8e4, tag="st3")

            # Byte-weighted round-robin across DVE/ACT (ACT is ~0.85x DVE)
            load = [0.0, 0.0]

            def evac(dst, ps, elems):
                if load[0] <= load[1] * 1.18:
                    nc.vector.tensor_copy(dst, ps)
                    load[0] += elems
                else:
                    nc.scalar.copy(dst, ps)
                    load[1] += elems

            # --- PE warm-up fillers (HAM un-throttles after ~3.4us busy)
            for i in range(14):
                psw = pps.tile([P, 512], dt.float32, tag="ps", name=f"w{i}")
                nc.tensor.matmul(psw[:, 0:P], warm_sb[:], warm_sb[:],
                                 start=True, stop=True)

            # --- t3: d=16, row-tiles at partitions 0-15 / 32-47; both
            # halves of a block share one 2-bank PSUM tile, single evac
            def t3_block(s):
                ps = ppb.tile([P, 2, 512], dt.float32, tag="pb",
                              name=f"ps3_{s}")
                for i, base in enumerate((0, 32)):
                    nc.tensor.matmul(
                        ps[:, i, 0:n3h],
                        in3_sb[base:base + 16, s * P:(s + 1) * P],
                        in3_sb[base:base + 16, 1024:1024 + n3h],
                        start=True, stop=True)
                evac(st3[:, s, :, :], ps[:, :, 0:n3h], 2 * n3h)

            # --- t0: d=1024, 8 accumulating k-tiles per dout block;
            # blocks 0/1 are split into k 0-3 / 4-7 phases to bridge the
            # in0 half-B arrival without stalling the in-order PE stream
            def p0_ap(k, s):
                base = (k // 4) * x0h
                return in0_sb[:, base + ((k % 4) * 4 + s) * P:
                              base + ((k % 4) * 4 + s + 1) * P]

            def e0_ap(k):
                base = (k // 4) * x0h + 2048
                return in0_sb[:, base + (k % 4) * n0q:
                              base + (k % 4 + 1) * n0q]

            ps0 = {}

            def t0_part(s, ks):
                ps = ps0.get(s)
                if ps is None:
                    ps = pps.tile([P, 512], dt.float32, tag="ps",
                                  name=f"ps0_{s}")
                    ps0[s] = ps
                for k in ks:
                    nc.tensor.matmul(ps[:, 0:n0q], p0_ap(k, s), e0_ap(k),
                                     start=(k == 0), stop=(k == 7))
                if ks[-1] == 7:
                    evac(st0[:, s, :], ps[:, 0:n0q], n0q)

            # --- t2: d=64, row-tiles at partitions 0-63 / 64-127; each
            # (block, half) gets a 2-bank PSUM tile, single 609-col evac
            def t2_block(s):
                for base, off in ((0, 0), (64, n2h)):
                    ps = ppb.tile([P, 1024], dt.float32, tag="pb",
                                  name=f"ps2_{s}_{base}")
                    for c0, c1 in _chunks(n2h):
                        nc.tensor.matmul(
                            ps[:, c0:c1],
                            in2_sb[base:base + 64, s * P:(s + 1) * P],
                            in2_sb[base:base + 64, 1024 + c0:1024 + c1],
                            start=True, stop=True)
                    evac(st2[:, s, off:off + n2h], ps[:, 0:n2h], n2h)

            def t1_block(s):
                ps = ppb.tile([P, 1024], dt.float32, tag="pb",
                              name=f"ps1_{s}")
                for k in range(2):
                    nc.tensor.matmul(
                        ps[:, 0:n1q],
                        in1_sb[:, (k * 4 + s) * P:(k * 4 + s + 1) * P],
                        in1_sb[:, 1024 + k * n1q:1024 + (k + 1) * n1q],
                        start=(k == 0), stop=(k == 1))
                evac(st1[:, s, :], ps[:, 0:n1q], n1q)

            # Interleave heavy-MM/light-evac t0 with light-MM/heavy-evac
            # t3/t2 so PSUM production stays under the DVE/ACT drain rate;
            # stores are split so the final one is small.
            for s in range(8):
                t3_block(s)
            nc.sync.dma_start(o3[:], st3[:])
            t0_part(0, [0, 1, 2, 3])
            t0_part(1, [0, 1, 2, 3])
            t2_block(0)
            t2_block(1)
            t0_part(0, [4, 5, 6, 7])
            t0_part(1, [4, 5, 6, 7])
            t2_block(2)
            t2_block(3)
            nc.sync.dma_start(o2[:, 0:4, :], st2[:, 0:4, :])
            t0_part(2, list(range(8)))
            t1_block(0)
            t1_block(1)
            t0_part(3, list(range(8)))
            nc.sync.dma_start(o0[:], st0[:])
            t1_block(2)
            t1_block(3)
            nc.sync.dma_start(o1[:], st1[:])
            t2_block(4)
            t2_block(5)
            nc.sync.dma_start(o2[:, 4:6, :], st2[:, 4:6, :])
            t2_block(6)
            t2_block(7)
            nc.sync.dma_start(o2[:, 6:8, :], st2[:, 6:8, :])

    nc.finalize()
    return nc


def _pad_cols(a, n):
    if a.shape[1] == n:
        return a
    out = np.zeros((a.shape[0], n), a.dtype)
    out[:, :a.shape[1]] = a
    return out


def kernel(inp, emb0, emb1, emb2, emb3, proj0, proj1, proj2, proj3):
    global LAST_RESULTS
    from concourse.bass_utils import run_bass_kernel_spmd

    flat = np.asarray(inp).reshape(-1).astype(np.int64)
    T = flat.shape[0]
    cuts = np.asarray(CUTS)
    tblid = np.searchsorted(cuts[1:], flat, side="right")
    embs = [np.asarray(e, np.float32) for e in (emb0, emb1, emb2, emb3)]
    projTs = [
        np.ascontiguousarray((np.asarray(p, np.float32) * EMB_SCALE).T)
        for p in (proj0, proj1, proj2, proj3)
    ]

    pos = {}
    loc = {}
    for t in range(4):
        pos[t] = np.nonzero(tblid == t)[0]
        loc[t] = flat[pos[t]] - cuts[t]

    n0q = max(1, -(-len(pos[0]) // 4))
    n1q = max(1, -(-len(pos[1]) // 4))
    n2h = max(1, -(-len(pos[2]) // 16))
    n3h = max(1, -(-len(pos[3]) // 16))
    n2c, n3c = 2 * n2h, 2 * n3h

    key = (n0q, n1q, n2h, n3h)
    nc = _PROGRAM_CACHE.get(key)
    if nc is None:
        nc = _build_program(*key)
        _PROGRAM_CACHE[key] = nc

    warm_np = np.zeros((P, P), BF16)

    e0_q = []
    for q in range(4):
        et = embs[0][loc[0][q::4]].T  # [1024, n]
        et = _pad_cols(et, n0q).reshape(8, P, n0q)
        e0_q.append(np.ascontiguousarray(et.transpose(1, 0, 2)).astype(BF16))
    pk0 = projTs[0].reshape(8, P, 8, P)  # [k, d_part, s_glob, c]
    p0_h = [
        np.ascontiguousarray(
            pk0[:, :, h * 4:(h + 1) * 4, :].transpose(1, 0, 2, 3)
        ).astype(BF16)
        for h in range(2)
    ]

    e1_q = []
    for q in range(4):
        et = embs[1][loc[1][q::4]].T  # [256, n]
        et = _pad_cols(et, n1q).reshape(2, P, n1q)
        e1_q.append(np.ascontiguousarray(et.transpose(1, 0, 2)).astype(BF16))
    pk1 = projTs[1].reshape(2, P, 8, P)
    p1_h = [
        np.ascontiguousarray(
            pk1[:, :, h * 4:(h + 1) * 4, :].transpose(1, 0, 2, 3)
        ).astype(BF16)
        for h in range(2)
    ]

    pk2 = projTs[2].reshape(64, 8 * P)
    p2 = np.concatenate([pk2, pk2], axis=0).astype(BF16)  # [128, 1024]
    pk3 = projTs[3].reshape(16, 8 * P)
    p3 = np.zeros((48, 8 * P), np.float32)
    p3[0:16] = pk3
    p3[32:48] = pk3
    p3 = p3.astype(BF16)

    in_maps = []
    core_meta = []
    for k in range(N_CORES):
        q, h = k // 2, k % 2

        p0 = p0_h[h]
        e0 = e0_q[q]
        in0 = np.concatenate([
            p0[:, 0:4].reshape(P, -1), e0[:, 0:4].reshape(P, -1),
            p0[:, 4:8].reshape(P, -1), e0[:, 4:8].reshape(P, -1),
        ], axis=1)

        in1 = np.concatenate([
            p1_h[h].reshape(P, -1), e1_q[q].reshape(P, -1)
        ], axis=1)

        rows2 = loc[2][k::8]
        nA2 = min(len(rows2), n2h)
        eA = _pad_cols(embs[2][rows2[:nA2]].T, n2h)
        eB = _pad_cols(embs[2][rows2[nA2:]].T, n2h)
        in2 = np.concatenate(
            [p2, np.concatenate([eA, eB], axis=0).astype(BF16)], axis=1)

        rows3 = loc[3][k::8]
        nA3 = min(len(rows3), n3h)
        e3 = np.zeros((48, n3h), np.float32)
        e3[0:16, :nA3] = embs[3][rows3[:nA3]].T
        e3[32:48, :len(rows3) - nA3] = embs[3][rows3[nA3:]].T
        in3 = np.concatenate([p3, e3.astype(BF16)], axis=1)

        in_maps.append({
            "warm": warm_np, "in0": np.ascontiguousarray(in0),
            "in1": np.ascontiguousarray(in1),
            "in2": np.ascontiguousarray(in2),
            "in3": np.ascontiguousarray(in3),
        })
        core_meta.append((nA2, nA3))

    trace = bool(os.environ.get("KERNEL_TRACE"))
    res = run_bass_kernel_spmd(nc, in_maps, core_ids=list(range(N_CORES)),
                               trace=trace)
    LAST_RESULTS = res

    out = np.empty((T, D_OUT), np.float32)

    for k in range(N_CORES):
        q, h = k // 2, k % 2
        r = res.results[k]
        for t, name in ((0, "o0"), (1, "o1")):
            positions = pos[t][q::4]
            n = len(positions)
            if n:
                dat = np.asarray(r[name])[:, :, :n].astype(np.float32)
                out[np.ix_(positions,
                           np.arange(h * 512, h * 512 + 512))] = (
                    dat.transpose(2, 1, 0).reshape(n, 512))
        nA2, nA3 = core_meta[k]
        p2k = pos[2][k::8]
        ob = np.asarray(r["o2"]).astype(np.float32)  # [128, 8, n2c]
        if nA2:
            out[p2k[:nA2]] = ob[:, :, :nA2].transpose(2, 1, 0).reshape(
                nA2, D_OUT)
        nB = len(p2k) - nA2
        if nB > 0:
            out[p2k[nA2:]] = ob[:, :, n2h:n2h + nB].transpose(2, 1, 0).reshape(
                nB, D_OUT)
        p3k = pos[3][k::8]
        ob = np.asarray(r["o3"]).astype(np.float32)  # [128, 8, 2, n3h]
        if nA3:
            out[p3k[:nA3]] = ob[:, :, 0, :nA3].transpose(2, 1, 0).reshape(
                nA3, D_OUT)
        nB = len(p3k) - nA3
        if nB > 0:
            out[p3k[nA3:]] = ob[:, :, 1, :nB].transpose(2, 1, 0).reshape(
                nB, D_OUT)

    return out.reshape(*np.asarray(inp).shape, D_OUT)
